# revision 20
# baseline (speedup 1.0000x reference)
"""Trainium2 Bass kernel for nn_Block (dense transformer block).

Shapes (hardcoded): x [8, 1024, 768], 12 heads x 64 head_dim, MLP hidden 16.
Sharding: data-parallel over batch, one batch element per NeuronCore (8 cores).

Device layout is feature-major ("transposed"): activations live as [feature,
token] tiles so every matmul contraction has its operand's contraction dim on
SBUF partitions.  The host pre-transposes x and pre-reorders the qkv weight
columns from the reference's interleaved (head_dim, head) order into
head-contiguous order, so head h occupies a contiguous 64-column block.

Numerics: matmuls run in float32r (full PE rate, ~1e-4 rel err); softmax
E=exp(S) and v are bf16; everything else fp32.
"""

import sys

for _p in ("/root/.axon_site", "/root/.axon_site/_ro/trn_rl_repo",
           "/root/.axon_site/_ro/pypackages", "/opt/trn_rl_repo"):
    if _p not in sys.path:
        sys.path.append(_p)

import numpy as np

import concourse.bacc as bacc
import concourse.tile as tile
import concourse.mybir as mybir
from concourse.bass_utils import run_bass_kernel_spmd

FP32 = mybir.dt.float32
FP32R = mybir.dt.float32r
BF16 = mybir.dt.bfloat16
AF = mybir.ActivationFunctionType
ALU = mybir.AluOpType

N_CORES = 8
D = 768          # model dim
P = 1024         # sequence length (tokens per core)
H = 12           # heads
HD = 64          # head dim
DT = D // 128    # feature tiles (6)
TT = P // 128    # token tiles (8)
MLP = 16
EPS = 1e-5
SCALE = HD ** -0.5


def _emit_stats(nc, psum, stats, sqp, src, ones128, eps_t):
    """LN statistics over features (partition axis), feature-major layout.

    Returns (negmu, sd, rstd): [1, 1024] fp32r rows.
      negmu = -mean(src, features);  sd = sqrt(var+eps);  rstd = 1/sd.
    The normalization itself is folded into downstream matmuls: with
    A = (x - mu) (x) g = x*g - g (x) mu, any W^T h = (W^T A)*rstd + W^T b
    becomes a matmul chain over x*g plus one K=1 row (W^T g)(x)(-mu), plus
    one K=1 row bias(x)sd (so the later *rstd restores the plain bias).
    """
    negmu = stats.tile([1, 1024], FP32R, tag="negmu", name="negmu")
    sd = stats.tile([1, 1024], FP32R, tag="sd", name="sd")
    rstd = stats.tile([1, 1024], FP32R, tag="rstd", name="rstd")
    m2_t = stats.tile([1, 1024], FP32, tag="m2_t", name="m2_t")
    tmp_t = stats.tile([1, 1024], FP32, tag="tmp_t", name="tmp_t")
    for hs in range(2):
        cs = slice(hs * 512, hs * 512 + 512)
        sum_ps = psum.tile([1, 512], FP32, tag="s", name="s")
        sum2_ps = psum.tile([1, 512], FP32, tag="s", name="s")
        for dt in range(DT):
            sq = sqp.tile([128, 512], FP32R, tag="sq", name="sq")
            nc.vector.tensor_mul(sq[:], src[dt][:, cs], src[dt][:, cs])
            nc.tensor.matmul(sum_ps[:], ones128[:], src[dt][:, cs],
                             start=(dt == 0), stop=(dt == DT - 1))
            nc.tensor.matmul(sum2_ps[:], ones128[:], sq[:],
                             start=(dt == 0), stop=(dt == DT - 1))
        m2, tmp = m2_t[:, cs], tmp_t[:, cs]
        nc.scalar.mul(negmu[:, cs], sum_ps[:], -1.0 / D)
        nc.scalar.mul(m2, sum2_ps[:], 1.0 / D)
        nc.vector.tensor_mul(tmp, negmu[:, cs], negmu[:, cs])   # mu^2
        nc.vector.tensor_sub(m2, m2, tmp)                       # var
        nc.scalar.activation(sd[:, cs], m2, AF.Sqrt, bias=eps_t[:])
        nc.vector.reciprocal(rstd[:, cs], sd[:, cs])
    return negmu, sd, rstd


def _emit(nc, tc, io):
    with nc.allow_low_precision(reason="fp32r/bf16 rounding fits error budget"), \
         tc.tile_pool(name="pers", bufs=1) as pers, \
         tc.tile_pool(name="psum", bufs=3, space="PSUM") as psum, \
         tc.tile_pool(name="psumo", bufs=2, space="PSUM") as psumo, \
         tc.tile_pool(name="stats", bufs=1) as stats, \
         tc.tile_pool(name="sqp", bufs=2) as sqp:

        # ---- constants (fp32r ones shipped from DRAM; memset can't write fp32r) ----
        ones128 = pers.tile([128, 1], FP32R, tag="ones128", name="ones128")
        nc.sync.dma_start(ones128[:], io["ones_col"][:])
        eps_t = pers.tile([1, 1], FP32, tag="eps", name="eps")
        nc.vector.memset(eps_t[:], EPS)

        rows = {}
        for nm, shp in (("wg_qk", [1, 2 * D]), ("b_qk_row", [1, 2 * D]),
                        ("wg_v", [1, D]), ("wg_fc1", [1, MLP]),
                        ("b_fc1_row", [1, MLP])):
            t = pers.tile(shp, FP32R, tag=nm, name=nm)
            nc.sync.dma_start(t[:], io[nm][:])
            rows[nm] = t
        g1_col = pers.tile([128, 6], FP32, tag="g1_col", name="g1_col")
        nc.sync.dma_start(g1_col[:], io["g1_col"][:])
        g2_col = pers.tile([128, 6], FP32, tag="g2_col", name="g2_col")
        nc.sync.dma_start(g2_col[:], io["g2_col"][:])
        b_proj = pers.tile([128, 6], FP32, tag="b_proj", name="b_proj")
        nc.sync.dma_start(b_proj[:], io["b_proj_col"][:])
        b_fc2 = pers.tile([128, 6], FP32, tag="b_fc2", name="b_fc2")
        nc.sync.dma_start(b_fc2[:], io["b_fc2_col"][:])

        # ---- persistent activation tiles ----
        v_aug = [pers.tile([128, H, HD + 1], BF16, tag=f"vaug{t}",
                           name=f"vaug{t}") for t in range(TT)]
        o_sb = [pers.tile([128, P], FP32R, tag=f"osb{i}", name=f"osb{i}")
                for i in range(DT)]
        out1 = [pers.tile([128, P], FP32R, tag=f"out1{i}", name=f"out1{i}")
                for i in range(DT)]

        with tc.tile_pool(name="phA", bufs=1) as phA:
            xg = [phA.tile([128, P], FP32R, tag=f"xg{dt}", name=f"xg{dt}")
                  for dt in range(DT)]

            # ======== LN1 stats + xg = x*g1 (x freed; reloaded for proj) ====
            with tc.tile_pool(name="xA", bufs=1) as xA:
                xT = []
                _eng = [nc.sync, nc.scalar, nc.gpsimd]
                for dt in range(DT):
                    t = xA.tile([128, P], FP32R, tag=f"xT{dt}", name=f"xT{dt}")
                    _eng[dt % 3].dma_start(t[:],
                                           io["xt"][dt * 128:(dt + 1) * 128, :])
                    xT.append(t)
                negmu, sd, rstd = _emit_stats(nc, psum, stats, sqp, xT,
                                              ones128, eps_t)
                for dt in range(DT):
                    nc.vector.tensor_scalar(xg[dt][:], xT[dt][:],
                                            g1_col[:, dt:dt + 1], None,
                                            op0=ALU.mult)
            # rstd broadcast along features (for q/k copies) and transposed
            # to column layout (for the token-major v copies)
            rstd_bc = phA.tile([128, P], FP32R, tag="rstd_bc", name="rstd_bc")
            nc.gpsimd.partition_broadcast(rstd_bc[:], rstd[:])
            # row -> column transpose of rstd via a DRAM bounce (DRAM APs
            # may be arbitrarily strided; SBUF partition-scatter DMAs may not)
            rstd_col = phA.tile([128, TT], FP32, tag="rstd_col",
                                name="rstd_col")
            with tc.tile_pool(name="drp", bufs=1, space="DRAM") as drp:
                rb = drp.tile([1, P], FP32, tag="rb", name="rb")
                nc.sync.dma_start(rb[:], rstd[0:1, :].bitcast(FP32))
                nc.sync.dma_start(rstd_col[:],
                                  rb.rearrange("o (to p) -> (o p) to", p=128))

            # ======== v = h @ w_v  (token-major, into v_aug) ========
            with tc.tile_pool(name="wv", bufs=1) as wvp:
                wv = wvp.tile([128, DT, D], FP32R, tag="wv", name="wv")
                _weng = [nc.gpsimd, nc.scalar, nc.sync]
                for dt in range(DT):
                    _weng[dt % 3].dma_start(
                        wv[:, dt, :], io["w_v"][:, dt * D:(dt + 1) * D])
                for t in range(TT):
                    tsl = slice(t * 128, (t + 1) * 128)
                    ps = psum.tile([128, 1024], FP32, tag="s", name="s")
                    for dt in range(DT):
                        nc.tensor.matmul(ps[:, 0:512], xg[dt][:, tsl],
                                         wv[:, dt, 0:512],
                                         start=(dt == 0), stop=False)
                        nc.tensor.matmul(ps[:, 512:768], xg[dt][:, tsl],
                                         wv[:, dt, 512:768],
                                         start=(dt == 0), stop=False)
                    nc.tensor.matmul(ps[:, 0:512], negmu[0:1, tsl],
                                     rows["wg_v"][0:1, 0:512],
                                     start=False, stop=True)
                    nc.tensor.matmul(ps[:, 512:768], negmu[0:1, tsl],
                                     rows["wg_v"][0:1, 512:768],
                                     start=False, stop=True)
                    nc.vector.memset(v_aug[t][:, :, HD:HD + 1], 1.0)
                    nc.vector.tensor_scalar(
                        v_aug[t][:, :, 0:HD],
                        ps[:, 0:768].rearrange("p (h d) -> p h d", d=HD),
                        rstd_col[:, t:t + 1], None, op0=ALU.mult)

            # ======== q,k (feature-major) + attention, per head-pair ========
            with tc.tile_pool(name="qk", bufs=6) as qkp, \
                 tc.tile_pool(name="wqk", bufs=3) as wqkp, \
                 tc.tile_pool(name="E", bufs=10) as ep, \
                 tc.tile_pool(name="bcp", bufs=2) as bcp, \
                 tc.tile_pool(name="rec", bufs=2) as recp:

                def emit_chain(hp):
                    qk_t = []
                    for blk in range(2):            # 0: q block, 1: k block
                        m = blk * 6 + hp
                        wm = wqkp.tile([128, DT, 128], FP32R, tag="wqk",
                                       name="wqk")
                        nc.sync.dma_start(
                            wm[:],
                            io["w_qk"][m].rearrange("p (o c) -> p o c", c=128))
                        msl = slice(m * 128, (m + 1) * 128)
                        ps = psum.tile([128, 1024], FP32, tag="s", name="s")
                        for dt in range(DT):
                            for hs in range(2):
                                cs = slice(hs * 512, hs * 512 + 512)
                                nc.tensor.matmul(ps[:, cs], wm[:, dt, :],
                                                 xg[dt][:, cs],
                                                 start=(dt == 0), stop=False)
                        for hs in range(2):
                            cs = slice(hs * 512, hs * 512 + 512)
                            nc.tensor.matmul(ps[:, cs], rows["wg_qk"][0:1, msl],
                                             negmu[0:1, cs],
                                             start=False, stop=False)
                            nc.tensor.matmul(ps[:, cs],
                                             rows["b_qk_row"][0:1, msl],
                                             sd[0:1, cs],
                                             start=False, stop=True)
                        qt = qkp.tile([128, P], FP32R, tag="qk", name="qk")
                        nc.vector.tensor_mul(qt[:], ps[:], rstd_bc[:])
                        qk_t.append(qt)
                    return qk_t

                chains = {0: emit_chain(0), 1: emit_chain(1)}
                for hp in range(6):
                    if hp + 2 < 6:
                        chains[hp + 2] = emit_chain(hp + 2)
                    qk_t = chains.pop(hp)

                    for hh in range(2):
                        h = 2 * hp + hh
                        pp = slice(hh * 64, hh * 64 + 64)
                        qh, kh = qk_t[0][pp, :], qk_t[1][pp, :]

                        # S^T[j,i] = sum_d k[j,d] q[i,d]; E = exp(S*scale)
                        e_tiles = []
                        o_ps = [psumo.tile([HD + 1, 512], FP32, tag="o",
                                           name="o") for _ in range(2)]
                        for j in range(TT):
                            s_ps = psum.tile([128, 1024], FP32, tag="s",
                                             name="s")
                            lhsT = kh[:, j * 128:(j + 1) * 128]
                            for hs in range(2):
                                cs = slice(hs * 512, hs * 512 + 512)
                                nc.tensor.matmul(s_ps[:, cs], lhsT, qh[:, cs],
                                                 start=True, stop=True)
                            ej = ep.tile([128, 1024], BF16, tag="E", name="E")
                            nc.scalar.activation(ej[:], s_ps[:], AF.Exp,
                                                 scale=SCALE)
                            e_tiles.append(ej)

                        # o~ = [v;1]^T @ E  (row 64 = softmax denominator)
                        for j in range(TT):
                            for hs in range(2):
                                cs = slice(hs * 512, hs * 512 + 512)
                                nc.tensor.matmul(o_ps[hs][:],
                                                 v_aug[j][:, h, :],
                                                 e_tiles[j][:, cs],
                                                 start=(j == 0),
                                                 stop=(j == TT - 1))

                        # normalize: o = o~ / r, per token half
                        for hs in range(2):
                            cs = slice(hs * 512, hs * 512 + 512)
                            rec = recp.tile([1, 512], FP32R, tag="rec",
                                            name="rec")
                            nc.vector.reciprocal(rec[:], o_ps[hs][HD:HD + 1, :])
                            bc_sb = bcp.tile([64, 512], FP32R, tag="bc",
                                             name="bc")
                            nc.gpsimd.partition_broadcast(bc_sb[:], rec[:])
                            nc.vector.tensor_mul(o_sb[h // 2][pp, cs],
                                                 o_ps[hs][0:HD, :], bc_sb[:])

        # ======== out1 = x + o @ w_proj + b_proj_eff ========
        with tc.tile_pool(name="wproj", bufs=3) as wpp, \
             tc.tile_pool(name="xB", bufs=1) as xB:
            for m in range(DT):
                xm = xB.tile([128, P], FP32R, tag=f"xTb{m}", name=f"xTb{m}")
                nc.scalar.dma_start(xm[:], io["xt"][m * 128:(m + 1) * 128, :])
                wm = wpp.tile([128, DT, 128], FP32R, tag="wproj", name="wproj")
                nc.gpsimd.dma_start(
                    wm[:],
                    io["w_proj"][m].rearrange("p (o c) -> p o c", c=128))
                ps = psum.tile([128, 1024], FP32, tag="s", name="s")
                for dt in range(DT):
                    for hs in range(2):
                        cs = slice(hs * 512, hs * 512 + 512)
                        nc.tensor.matmul(ps[:, cs], wm[:, dt, :],
                                         o_sb[dt][:, cs],
                                         start=(dt == 0), stop=(dt == DT - 1))
                nc.vector.scalar_tensor_tensor(out1[m][:], ps[:],
                                               b_proj[:, m:m + 1], xm[:],
                                               op0=ALU.add, op1=ALU.add)

        # ======== MLP branch ========
        with tc.tile_pool(name="phC", bufs=1) as phC, \
             tc.tile_pool(name="outp", bufs=3) as outp:
            negmu2, sd2, rstd2 = _emit_stats(nc, psum, stats, sqp, out1,
                                             ones128, eps_t)
            xg2 = [phC.tile([128, P], FP32R, tag=f"xg2{dt}", name=f"xg2{dt}")
                   for dt in range(DT)]
            for dt in range(DT):
                nc.vector.tensor_scalar(xg2[dt][:], out1[dt][:],
                                        g2_col[:, dt:dt + 1], None,
                                        op0=ALU.mult)
            rstd2_bc = phC.tile([MLP, P], FP32R, tag="rstd2_bc",
                                name="rstd2_bc")
            nc.gpsimd.partition_broadcast(rstd2_bc[:], rstd2[:])

            wf1 = phC.tile([128, DT, MLP], FP32R, tag="wfc1", name="wfc1")
            nc.sync.dma_start(
                wf1[:], io["w_fc1"].rearrange("p (o c) -> p o c", c=MLP))
            wf2 = phC.tile([MLP, D], FP32R, tag="wfc2", name="wfc2")
            nc.sync.dma_start(wf2[:], io["w_fc2"][:])

            g_ps = psum.tile([MLP, 1024], FP32, tag="s", name="s")
            for dt in range(DT):
                for hs in range(2):
                    cs = slice(hs * 512, hs * 512 + 512)
                    nc.tensor.matmul(g_ps[:, cs], wf1[:, dt, :],
                                     xg2[dt][:, cs],
                                     start=(dt == 0), stop=False)
            for hs in range(2):
                cs = slice(hs * 512, hs * 512 + 512)
                nc.tensor.matmul(g_ps[:, cs], rows["wg_fc1"][0:1, :],
                                 negmu2[0:1, cs], start=False, stop=False)
                nc.tensor.matmul(g_ps[:, cs], rows["b_fc1_row"][0:1, :],
                                 sd2[0:1, cs], start=False, stop=True)
            gpre = phC.tile([MLP, P], FP32, tag="gpre", name="gpre")
            nc.vector.tensor_mul(gpre[:], g_ps[:], rstd2_bc[:])
            gact = phC.tile([MLP, 1024], FP32R, tag="gact", name="gact")
            nc.scalar.activation(gact[:], gpre[:], AF.Gelu)

            for m in range(DT):
                ps = psum.tile([128, 1024], FP32, tag="s", name="s")
                for hs in range(2):
                    cs = slice(hs * 512, hs * 512 + 512)
                    nc.tensor.matmul(ps[:, cs], wf2[:, m * 128:(m + 1) * 128],
                                     gact[:, cs], start=True, stop=True)
                ot = outp.tile([128, P], FP32, tag="outT", name="outT")
                nc.vector.scalar_tensor_tensor(ot[:], ps[:], b_fc2[:, m:m + 1],
                                               out1[m][:], op0=ALU.add,
                                               op1=ALU.add)
                nc.scalar.dma_start(io["out"][m * 128:(m + 1) * 128, :], ot[:])


def build():
    nc = bacc.Bacc("TRN2", target_bir_lowering=False, debug=False,
                   num_devices=N_CORES)
    io = {
        "xt": nc.dram_tensor("xt", [D, P], FP32R, kind="ExternalInput").ap(),
        "w_qk": nc.dram_tensor("w_qk", [12, 128, DT * 128], FP32R,
                               kind="ExternalInput").ap(),
        "w_v": nc.dram_tensor("w_v", [128, DT * D], FP32R,
                              kind="ExternalInput").ap(),
        "w_proj": nc.dram_tensor("w_proj", [DT, 128, DT * 128], FP32R,
                                 kind="ExternalInput").ap(),
        "w_fc1": nc.dram_tensor("w_fc1", [128, DT * MLP], FP32R,
                                kind="ExternalInput").ap(),
        "w_fc2": nc.dram_tensor("w_fc2", [MLP, D], FP32R,
                                kind="ExternalInput").ap(),
        "ones_col": nc.dram_tensor("ones_col", [128, 1], FP32R,
                                   kind="ExternalInput").ap(),
        "wg_qk": nc.dram_tensor("wg_qk", [1, 2 * D], FP32R,
                                kind="ExternalInput").ap(),
        "b_qk_row": nc.dram_tensor("b_qk_row", [1, 2 * D], FP32R,
                                   kind="ExternalInput").ap(),
        "wg_v": nc.dram_tensor("wg_v", [1, D], FP32R,
                               kind="ExternalInput").ap(),
        "wg_fc1": nc.dram_tensor("wg_fc1", [1, MLP], FP32R,
                                 kind="ExternalInput").ap(),
        "b_fc1_row": nc.dram_tensor("b_fc1_row", [1, MLP], FP32R,
                                    kind="ExternalInput").ap(),
        "g1_col": nc.dram_tensor("g1_col", [128, 6], FP32,
                                 kind="ExternalInput").ap(),
        "g2_col": nc.dram_tensor("g2_col", [128, 6], FP32,
                                 kind="ExternalInput").ap(),
        "b_proj_col": nc.dram_tensor("b_proj_col", [128, 6], FP32,
                                     kind="ExternalInput").ap(),
        "b_fc2_col": nc.dram_tensor("b_fc2_col", [128, 6], FP32,
                                    kind="ExternalInput").ap(),
        "out": nc.dram_tensor("out", [D, P], FP32, kind="ExternalOutput").ap(),
    }
    with tile.TileContext(nc) as tc:
        _emit(nc, tc, io)
    nc.compile()
    return nc


def prep_inputs(x, g1, b1, w_qkv, b_qkv, w_proj, b_proj, g2, b2,
                w_fc1, b_fc1, w_fc2, b_fc2):
    """Host-side re-layout of the full inputs into per-core in_maps."""
    f32 = np.float32
    asf = lambda a: np.ascontiguousarray(a, dtype=f32)

    # reference splits the 2304 qkv dim as (3, head_dim=64, heads=12);
    # reorder columns to (3, heads, head_dim) so heads are contiguous.
    i3, d, h = np.meshgrid(np.arange(3), np.arange(HD), np.arange(H),
                           indexing="ij")
    perm = (i3 * D + d * H + h).reshape(3, HD, H).transpose(0, 2, 1).reshape(-1)
    w_re = np.asarray(w_qkv, dtype=f32)[:, perm]
    b_re = np.asarray(b_qkv, dtype=f32)[perm]

    w_proj = np.asarray(w_proj, dtype=f32)
    g1 = np.asarray(g1, f32); b1 = np.asarray(b1, f32)
    g2 = np.asarray(g2, f32); b2 = np.asarray(b2, f32)
    w_fc1 = np.asarray(w_fc1, f32); w_fc2 = np.asarray(w_fc2, f32)
    w_qk = w_re[:, :2 * D]
    w_v = w_re[:, 2 * D:]
    b_v_total = b_re[2 * D:] + w_v.T @ b1
    b_proj_eff = np.asarray(b_proj, dtype=f32) + b_v_total @ w_proj

    common = {
        "ones_col": np.ones((128, 1), f32),
        # weights pre-tiled so each DMA is contiguous per partition:
        # w_qk[m][p][o*128+c] = w_qk[o*128+p, m*128+c], etc.
        "w_qk": asf(w_qk.reshape(6, 128, 12, 128).transpose(2, 1, 0, 3)
                    .reshape(12, 128, 768)),
        "w_v": asf(w_v.reshape(6, 128, D).transpose(1, 0, 2)
                   .reshape(128, 6 * D)),
        "w_proj": asf(w_proj.reshape(6, 128, 6, 128).transpose(2, 1, 0, 3)
                      .reshape(6, 128, 768)),
        "w_fc1": asf(w_fc1.reshape(6, 128, MLP).transpose(1, 0, 2)
                     .reshape(128, 6 * MLP)),
        "w_fc2": asf(w_fc2),
        "wg_qk": asf((w_qk.T @ g1).reshape(1, -1)),
        "b_qk_row": asf((b_re[:2 * D] + w_qk.T @ b1).reshape(1, -1)),
        "wg_v": asf((w_v.T @ g1).reshape(1, -1)),
        "wg_fc1": asf((w_fc1.T @ g2).reshape(1, -1)),
        "b_fc1_row": asf((np.asarray(b_fc1, f32) + w_fc1.T @ b2)
                         .reshape(1, -1)),
        "g1_col": asf(g1.reshape(6, 128).T),
        "g2_col": asf(g2.reshape(6, 128).T),
        "b_proj_col": asf(b_proj_eff.reshape(6, 128).T),
        "b_fc2_col": asf(np.asarray(b_fc2, f32).reshape(6, 128).T),
    }
    x = np.asarray(x, dtype=f32)
    return [dict(common, xt=asf(x[i].T)) for i in range(N_CORES)]


_NC_CACHE = None


def kernel(**inputs):
    global _NC_CACHE
    if _NC_CACHE is None:
        _NC_CACHE = build()
    in_maps = prep_inputs(**inputs)
    res = run_bass_kernel_spmd(_NC_CACHE, in_maps, list(range(N_CORES)))
    return np.stack([res.results[i]["out"].T for i in range(N_CORES)])
